# revision 1
# baseline (speedup 1.0000x reference)
"""Trainium2 Bass kernel for ClassicPINN forward pass (15-layer tiny MLP, tanh).

Strategy
--------
Pure data parallel over 8 NeuronCores (131072 points each). Within a core,
points are processed in 4 chunks of 32768. Activations live feature-on-
partition: the 128 SBUF partitions hold G groups of the layer width, each
group handling a different 512-point column block. Layer weights are
block-diagonalized on the host (G copies of the tiny W^T along the
diagonal) so every matmul is a dense [K<=128, M<=128] x [K, 512] -> PSUM.
Four matmuls fill a 4-bank PSUM tile [128, 2048]; one ScalarE ACTIVATE
per PSUM tile applies tanh(x + b) (bias as a per-partition AP) and writes
SBUF, amortizing the ~352-cycle ACT overhead. ACT is the roofline here
(~216 tanh elements/point); PE, DMA and DVE all hide under it.

The same schedule object drives the Bass builder, a numpy simulator
(used by test.py), and an integer "point id" replay that yields the
output unpack permutation.
"""

import numpy as np
from contextlib import ExitStack

WIDTHS = [3, 8, 8, 8, 8, 8, 8, 8, 16, 16, 16, 32, 32, 32, 16, 3]
N_LAYERS = 15
N_POINTS = 1048576
NCORES = 8
PPC = N_POINTS // NCORES          # 131072 points per core
NCHUNKS = 4
CHUNK = PPC // NCHUNKS            # 32768 points per chunk
FREE = 512                        # matmul moving free dim (fp32 max)
ACT_BANKS = 4                     # PSUM banks per ACTIVATE (4*512 cols)
MM_DTYPE = "tf32"                 # "f32" (exact, 4 cyc/row) | "tf32" (1 cyc/row)


class _Layer:
    pass


def _make_schedule():
    """Per-layer matmul/activation schedule for one 32768-point chunk."""
    layers = []
    cur_groups = 16               # coords: 16 groups of 3 features = 48 partitions
    w_off = 0
    for i in range(N_LAYERS):
        in_w, out_w = WIDTHS[i], WIDTHS[i + 1]
        out_w_pad = 8 if out_w == 3 else out_w   # pad final width 3 -> 8
        L = _Layer()
        L.i = i
        L.in_w, L.out_w, L.out_w_pad = in_w, out_w, out_w_pad
        L.Gmm = min(cur_groups, 128 // in_w, 128 // out_w_pad)
        L.Kmm = L.Gmm * in_w
        L.Mmm = L.Gmm * out_w_pad
        L.n_half = cur_groups // L.Gmm           # input partition slices
        # fp32r matmuls must write PSUM base partition 0, so never stack
        # multiple matmuls partition-wise in a bank. Short (Mmm<128) PSUM
        # tiles are re-folded to 128 partitions on the ACT/DVE drain, which
        # has no dst-partition restriction: psum tile t lands on row block
        # Mmm*(t % fold) of a [128, out_cols/fold] SBUF tile.
        L.pack = 1
        L.out_height = L.Mmm
        L.fold = 128 // L.Mmm
        L.in_groups = cur_groups
        L.in_cols = CHUNK // cur_groups
        ncb_in = L.in_cols // FREE
        L.n_mms = ncb_in * L.n_half
        L.out_groups = L.Gmm
        L.out_cols = CHUNK // L.out_groups
        L.n_ocb = L.n_mms                        # 512-col output blocks
        L.n_psum = (L.n_ocb + ACT_BANKS - 1) // ACT_BANKS
        L.w_off = w_off
        w_off += L.Mmm
        L.mms = []
        for h in range(L.n_half):
            for cb in range(ncb_in):
                m = h * ncb_in + cb
                L.mms.append(dict(
                    rhs_p0=h * L.Kmm,
                    rhs_c0=cb * FREE,
                    out_p0=0,
                    ocb=m,
                ))
        L.out_cols_phys = L.out_cols // L.fold
        # lhsT must sit at the same SBUF base partition as the rhs it is
        # multiplied with; enumerate every physical rhs base this layer uses
        # (input-half slices x input fold blocks) and stack the block-diag
        # weights at each.
        if i == 0:
            pm, pf = 48, 1
        else:
            pm, pf = layers[i - 1].Mmm, layers[i - 1].fold
        L.rhs_bases = sorted({pm * k + h * L.Kmm
                              for k in range(pf) for h in range(L.n_half)})
        layers.append(L)
        cur_groups = L.out_groups
    return layers, w_off


_BLK = ACT_BANKS * FREE           # virtual cols per PSUM tile / fold block


def _phys_out(L, ocb):
    """Physical (row0, col0) in the folded SBUF tile for virtual 512-col
    output block `ocb` of layer L."""
    t = ocb // ACT_BANKS
    return L.Mmm * (t % L.fold), (t // L.fold) * _BLK + (ocb % ACT_BANKS) * FREE


def _phys_in(L, p0, vc0):
    """Physical (row0, col0) for reading K rows at virtual (p0, vc0) from
    layer L's input tile (the previous layer's folded output)."""
    if L.i == 0:
        return p0, vc0
    prev = _LAYERS[L.i - 1]
    t = vc0 // _BLK
    return (prev.Mmm * (t % prev.fold) + p0,
            (t // prev.fold) * _BLK + vc0 % _BLK)


_LAYERS, W_TOTAL = _make_schedule()


# ---------------------------------------------------------------- host packing

def pack_coords(coords):
    """[N_POINTS, 3] -> [NCORES, NCHUNKS, 48, 2048] matching the L0 layout.

    Per core: point n = chunk*32768 + t*8192 + g*512 + j lives at
    partition g*3+f, column t*512+j of tile [core, chunk].
    """
    c = np.ascontiguousarray(coords, dtype=np.float32)
    c = c.reshape(NCORES, NCHUNKS, 4, 16, FREE, 3)
    c = c.transpose(0, 1, 3, 5, 2, 4)            # core, chunk, g, f, t, j
    return np.ascontiguousarray(c.reshape(NCORES, NCHUNKS, 48, 2048))


def build_weights(Ws, bs):
    """Block-diagonal lhsT stack [128, W_TOTAL] and bias matrix [128, 15]."""
    lhsT_all = np.zeros((128, W_TOTAL), np.float32)
    biases = np.zeros((128, N_LAYERS), np.float32)
    for L in _LAYERS:
        W = np.asarray(Ws[L.i], np.float32)      # [out_w, in_w]
        bd = np.zeros((L.Kmm, L.Mmm), np.float32)
        for g in range(L.Gmm):
            bd[g * L.in_w:(g + 1) * L.in_w,
               g * L.out_w_pad:g * L.out_w_pad + L.out_w] = W.T
        for base in L.rhs_bases:
            lhsT_all[base:base + L.Kmm, L.w_off:L.w_off + L.Mmm] = bd
        b = np.asarray(bs[L.i], np.float32)
        q = np.arange(128) % L.out_w_pad
        col = np.where(q < L.out_w, b[np.minimum(q, L.out_w - 1)], 0.0)
        biases[:, L.i] = col
    return lhsT_all, biases


def replay_ids():
    """Propagate chunk-local point ids through the schedule.

    Returns [128, out_cols] int array: element (p, c) of the final output
    tile holds component (p % out_w_pad) of chunk-local point ids[p, c].
    """
    ids = np.zeros((48, 2048), np.int64)
    j = np.arange(FREE)
    for g in range(16):
        for t in range(4):
            for f in range(3):
                ids[g * 3 + f, t * FREE:(t + 1) * FREE] = t * 8192 + g * FREE + j
    for L in _LAYERS:
        out = np.zeros((L.Mmm * L.fold, L.out_cols_phys), np.int64)
        for mm in L.mms:
            pr, pc = _phys_in(L, mm['rhs_p0'], mm['rhs_c0'])
            src = ids[pr:pr + L.Kmm:L.in_w, pc:pc + FREE]      # [Gmm, 512]
            orr, occ = _phys_out(L, mm['ocb'])
            out[orr:orr + L.Mmm, occ:occ + FREE] = \
                np.repeat(src, L.out_w_pad, axis=0)
        ids = out
    return ids


def simulate_chunk(coords_tile, lhsT_all, biases):
    """Numpy mirror of the device program for one [48, 2048] chunk tile."""
    act = coords_tile.astype(np.float32)
    for L in _LAYERS:
        H = L.Mmm * L.fold
        out = np.zeros((H, L.out_cols_phys), np.float32)
        for mm in L.mms:
            pr, pc = _phys_in(L, mm['rhs_p0'], mm['rhs_c0'])
            lhsT = lhsT_all[pr:pr + L.Kmm, L.w_off:L.w_off + L.Mmm]
            rhs = act[pr:pr + L.Kmm, pc:pc + FREE]
            orr, occ = _phys_out(L, mm['ocb'])
            out[orr:orr + L.Mmm, occ:occ + FREE] = lhsT.T @ rhs
        out += biases[:H, L.i:L.i + 1]
        act = np.tanh(out) if L.i < N_LAYERS - 1 else out
    return act                                   # [128, out_cols_phys]


def unpack_output(per_core_out):
    """[NCORES][NCHUNKS, 128, out_cols] device tiles -> [N_POINTS, 3]."""
    ids = replay_ids()
    rows = np.arange(ids.shape[0])
    comp = rows % _LAYERS[-1].out_w_pad
    valid = comp < 3
    n_idx = ids[valid]
    o_idx = np.broadcast_to(comp[valid][:, None], n_idx.shape)
    out = np.empty((N_POINTS, 3), np.float32)
    for core in range(NCORES):
        tiles = per_core_out[core]
        for chunk in range(NCHUNKS):
            base = core * PPC + chunk * CHUNK
            out[base + n_idx, o_idx] = tiles[chunk][valid]
    return out


# ---------------------------------------------------------------- bass program

_PROGRAM_CACHE = {}


def _build_program(repeat=1):
    import concourse.bacc as bacc
    import concourse.bass as bass
    import concourse.tile as tile
    from concourse import mybir

    nc = bacc.Bacc("TRN2", target_bir_lowering=False, debug=False,
                   enable_asserts=False, num_devices=NCORES)
    dt = mybir.dt.float32
    # float32r is TF32 (10-bit mantissa) streamed at 1 cycle/row through the
    # PE vs 4 for exact fp32. The whole matmul dataflow (coords, weights,
    # activations) must be typed float32r so the BIR verifier sees rounded
    # producers; PSUM accumulation and the bias path stay fp32.
    mdt = mybir.dt.float32r if MM_DTYPE == "tf32" else dt
    coords_d = nc.dram_tensor("coords", (NCHUNKS, 48, 2048), mdt,
                              kind="ExternalInput").ap()
    w_d = nc.dram_tensor("lhsT_all", (128, W_TOTAL), mdt,
                         kind="ExternalInput").ap()
    b_d = nc.dram_tensor("biases", (128, N_LAYERS), dt,
                         kind="ExternalInput").ap()
    Lf = _LAYERS[-1]
    out_d = nc.dram_tensor(
        "out", (NCHUNKS, Lf.Mmm * Lf.fold, Lf.out_cols_phys), dt,
        kind="ExternalOutput").ap()

    TANH = mybir.ActivationFunctionType.Tanh

    with tile.TileContext(nc) as tc, ExitStack() as ctx:
        wpool = ctx.enter_context(tc.tile_pool(name="weights", bufs=1))
        cpool = ctx.enter_context(tc.tile_pool(name="cin", bufs=2))
        a8 = ctx.enter_context(tc.tile_pool(name="a8", bufs=4))
        a16 = ctx.enter_context(tc.tile_pool(name="a16", bufs=4))
        a32 = ctx.enter_context(tc.tile_pool(name="a32", bufs=2))
        pspool = ctx.enter_context(
            tc.tile_pool(name="psum", bufs=2, space="PSUM"))

        wt = wpool.tile([128, W_TOTAL], mdt, tag="wt")
        nc.sync.dma_start(out=wt[:], in_=w_d[:])
        bt = wpool.tile([128, N_LAYERS], dt, tag="bt")
        nc.sync.dma_start(out=bt[:], in_=b_d[:])

        pool_by_cols = {2048: a8, 4096: a16, 8192: a32}

        def emit_layer(L, act):
            pool = pool_by_cols[L.out_cols_phys]
            is_last = L.i == N_LAYERS - 1
            out_t = pool.tile([L.Mmm * L.fold, L.out_cols_phys],
                              dt if is_last else mdt, tag=pool.name)
            for t in range(L.n_psum):
                banks = min(ACT_BANKS, L.n_ocb - ACT_BANKS * t)
                ps = pspool.tile([L.Mmm, banks * FREE], dt, tag="ps")
                for mm in L.mms[t * ACT_BANKS:t * ACT_BANKS + banks]:
                    lc = (mm['ocb'] - ACT_BANKS * t) * FREE
                    pr, pc = _phys_in(L, mm['rhs_p0'], mm['rhs_c0'])
                    nc.tensor.matmul(
                        ps[0:L.Mmm, lc:lc + FREE],
                        wt[pr:pr + L.Kmm, L.w_off:L.w_off + L.Mmm],
                        act[pr:pr + L.Kmm, pc:pc + FREE],
                        start=True, stop=True)
                r0 = L.Mmm * (t % L.fold)
                c0 = (t // L.fold) * _BLK
                dst = out_t[r0:r0 + L.Mmm, c0:c0 + banks * FREE]
                if is_last:
                    # Final layer: bias-add on the otherwise-idle DVE, so
                    # ACT never leaves the Tanh table set.
                    nc.vector.tensor_scalar_add(
                        dst, ps[:], bt[r0:r0 + L.Mmm, L.i:L.i + 1])
                else:
                    nc.scalar.activation(
                        dst, ps[:], TANH, bias=bt[r0:r0 + L.Mmm, L.i:L.i + 1])
            return out_t

        # Chunks run in interleaved pairs through the narrow (1-2 ACTs per
        # layer) early layers so ACT ping-pongs between the two chunks'
        # PSUM tiles; the wide layers (>=4 PSUM tiles each) self-pipeline
        # and run per-chunk to fit SBUF.
        seq = [c for _ in range(repeat) for c in range(NCHUNKS)]
        N_PAIRED = 10
        for ca, cb in zip(seq[0::2], seq[1::2]):
            acts = {}
            for c in (ca, cb):
                ct = cpool.tile([48, 2048], mdt, tag="cin")
                nc.sync.dma_start(out=ct[:], in_=coords_d[c])
                acts[c] = ct
            for L in _LAYERS[:N_PAIRED]:
                for c in (ca, cb):
                    acts[c] = emit_layer(L, acts[c])
            for c in (ca, cb):
                act = acts[c]
                for L in _LAYERS[N_PAIRED:]:
                    act = emit_layer(L, act)
                nc.sync.dma_start(out=out_d[c], in_=act[:])

    nc.compile()
    return nc


def get_program(repeat=1):
    key = ("nc", repeat)
    if key not in _PROGRAM_CACHE:
        _PROGRAM_CACHE[key] = _build_program(repeat)
    return _PROGRAM_CACHE[key]


def make_in_maps(coords, Ws, bs):
    cp = pack_coords(coords)
    lhsT_all, biases = build_weights(Ws, bs)
    return [{"coords": cp[core], "lhsT_all": lhsT_all, "biases": biases}
            for core in range(NCORES)]


def kernel(**inputs):
    from concourse.bass_utils import run_bass_kernel_spmd

    coords = np.asarray(inputs["coords"], np.float32)
    Ws = [np.asarray(inputs[f"W{i}"], np.float32) for i in range(N_LAYERS)]
    bs = [np.asarray(inputs[f"b{i}"], np.float32) for i in range(N_LAYERS)]

    nc = get_program()
    in_maps = make_in_maps(coords, Ws, bs)
    res = run_bass_kernel_spmd(nc, in_maps, list(range(NCORES)))
    per_core = [res.results[c]["out"] for c in range(NCORES)]
    full = unpack_output(per_core)
    return (full[:, 0:1], full[:, 1:2], full[:, 2:3])



# revision 9
# speedup vs baseline: 1.1714x; 1.1714x over previous
"""Trainium2 Bass kernel for ClassicPINN forward pass (15-layer tiny MLP, tanh).

Strategy (v2)
-------------
Pure data parallel over 8 NeuronCores (131072 points each); 4 chunks of
32768 points per core. Activations live feature-on-partition in bf16:
the 128 SBUF partitions hold G groups of the layer width, each group a
different 512-point column block. Weights are block-diagonalized on the
host so every matmul is dense [K<=128, M<=128] x [K, 512] -> PSUM fp32.

ACT (ScalarE tanh) is the roofline (~216 tanh elems/point). v2 cuts ACT
waste and idle vs v1:
  * bf16 matmul dataflow (same 1 cyc/row PE speed as fp32r) allows PE
    tile_position, so layers with Mmm=64 (L13, L14) stack 2 matmuls
    partition-wise in one PSUM bank set -> every ACT drains a full
    [128, 2048] tile (v1 ran half-width ACTs for L13).
  * L14 reads the *folded* L13 tile with Gmm=8 (was 4): half the
    matmuls and half the DVE drain of v1.
  * All 15 layers run in interleaved chunk pairs (bf16 halves SBUF so
    the wide layers fit); each pair's L14 (no tanh -> no ACT work) is
    deferred into the next pair's early layers so ACT never waits on a
    pair transition. The final pair interleaves L13/L14 per chunk.
  * Coords prefetched a pair ahead; weight DMA split so L0/L1 weights
    land before the bulk; bf16 output halves the out-DMA.

The same schedule object drives the Bass builder, a numpy simulator
(used by test.py), and an integer "point id" replay that yields the
output unpack permutation.
"""

import numpy as np
from contextlib import ExitStack
import ml_dtypes

BF16 = ml_dtypes.bfloat16

WIDTHS = [3, 8, 8, 8, 8, 8, 8, 8, 16, 16, 16, 32, 32, 32, 16, 3]
N_LAYERS = 15
N_POINTS = 1048576
NCORES = 8
PPC = N_POINTS // NCORES          # 131072 points per core
NCHUNKS = 4
CHUNK = PPC // NCHUNKS            # 32768 points per chunk
FREE = 512                        # matmul moving free dim
BANKS = 4                         # PSUM banks per mega-tile / ACT
BLK = BANKS * FREE                # 2048 cols per mega-tile
WT_HEAD_LAYERS = 2                # layers whose weights ship in the early DMA


class _Layer:
    pass


def _make_schedule():
    """Uniform per-layer schedule. Every layer's input is a physical
    [P_in, C_in] tile holding G_in groups of in_w features per column
    (stacked layers interleave two virtual column blocks row-wise, which
    simply doubles the group count seen by the next layer)."""
    layers = []
    G_in, C_in, P_in = 16, 2048, 48   # coords: 16 groups x 3 feats
    w_off = 0
    for i in range(N_LAYERS):
        in_w, out_w = WIDTHS[i], WIDTHS[i + 1]
        out_w_pad = 8 if out_w == 3 else out_w
        L = _Layer()
        L.i, L.in_w, L.out_w, L.out_w_pad = i, in_w, out_w, out_w_pad
        L.G_in, L.C_in, L.P_in = G_in, C_in, P_in
        L.Gmm = min(G_in, 128 // out_w_pad)
        L.n_half = G_in // L.Gmm
        L.Kmm = L.Gmm * in_w
        L.Mmm = L.Gmm * out_w_pad
        L.stack = 128 // L.Mmm        # matmuls stacked per PSUM bank set
        L.ncb = C_in // FREE
        L.n_mms = L.ncb * L.n_half
        assert L.n_mms % (BANKS * L.stack) == 0, (i, L.n_mms, L.stack)
        L.n_mega = L.n_mms // (BANKS * L.stack)
        L.C_out = L.n_mega * BLK
        L.w_off = w_off
        w_off += L.Mmm
        layers.append(L)
        G_in, C_in, P_in = 128 // out_w_pad, L.C_out, 128
    return layers, w_off


_LAYERS, W_TOTAL = _make_schedule()
W_HEAD = sum(L.Mmm for L in _LAYERS[:WT_HEAD_LAYERS])


def _mm_geom(L, m):
    """Matmul m -> (input half, input col block, mega-tile, stack row, bank)."""
    h, cbi = divmod(m, L.ncb)
    tau, r = divmod(m, BANKS * L.stack)
    s, b = divmod(r, BANKS)
    return h, cbi, tau, s, b


# ---------------------------------------------------------------- host packing

def pack_coords(coords):
    """[N_POINTS, 3] -> bf16 [NCORES, NCHUNKS, 48, 2048] matching L0 layout.

    Per core: point n = chunk*32768 + t*8192 + g*512 + j lives at
    partition g*3+f, column t*512+j of tile [core, chunk].
    """
    c = np.ascontiguousarray(coords, dtype=np.float32)
    c = c.reshape(NCORES, NCHUNKS, 4, 16, FREE, 3)
    c = c.transpose(0, 1, 3, 5, 2, 4)            # core, chunk, g, f, t, j
    return np.ascontiguousarray(c.reshape(NCORES, NCHUNKS, 48, 2048)).astype(BF16)


def build_weights(Ws, bs):
    """Block-diagonal lhsT stacks (bf16 head/rest) and bias matrix [128, 15]."""
    lhsT_all = np.zeros((128, W_TOTAL), np.float32)
    biases = np.zeros((128, N_LAYERS), np.float32)
    for L in _LAYERS:
        W = np.asarray(Ws[L.i], np.float32)      # [out_w, in_w]
        bd = np.zeros((L.Kmm, L.Mmm), np.float32)
        for g in range(L.Gmm):
            bd[g * L.in_w:(g + 1) * L.in_w,
               g * L.out_w_pad:g * L.out_w_pad + L.out_w] = W.T
        for h in range(L.n_half):
            base = h * L.Kmm
            lhsT_all[base:base + L.Kmm, L.w_off:L.w_off + L.Mmm] = bd
        b = np.asarray(bs[L.i], np.float32)
        q = np.arange(128) % L.out_w_pad
        col = np.where(q < L.out_w, b[np.minimum(q, L.out_w - 1)], 0.0)
        biases[:, L.i] = col
    lhsT16 = lhsT_all.astype(BF16)
    return lhsT16[:, :W_HEAD].copy(), lhsT16[:, W_HEAD:].copy(), biases


def replay_ids():
    """Propagate chunk-local point ids through the schedule.

    Returns [128, 2048] int array: element (p, c) of the final output
    tile holds component (p % 8) of chunk-local point ids[p, c].
    """
    ids = np.zeros((48, 2048), np.int64)
    j = np.arange(FREE)
    for g in range(16):
        for t in range(4):
            for f in range(3):
                ids[g * 3 + f, t * FREE:(t + 1) * FREE] = t * 8192 + g * FREE + j
    for L in _LAYERS:
        out = np.zeros((128, L.C_out), np.int64)
        for m in range(L.n_mms):
            h, cbi, tau, s, b = _mm_geom(L, m)
            src = ids[h * L.Kmm:h * L.Kmm + L.Kmm:L.in_w,
                      cbi * FREE:(cbi + 1) * FREE]          # [Gmm, 512]
            out[s * L.Mmm:(s + 1) * L.Mmm,
                tau * BLK + b * FREE:tau * BLK + (b + 1) * FREE] = \
                np.repeat(src, L.out_w_pad, axis=0)
        ids = out
    return ids


def simulate_chunk(coords_tile, lhsT_head, lhsT_rest, biases):
    """Numpy mirror of the device program for one [48, 2048] chunk tile."""
    lhsT_all = np.concatenate([np.asarray(lhsT_head, np.float32),
                               np.asarray(lhsT_rest, np.float32)], axis=1)
    act = np.asarray(coords_tile, np.float32)
    for L in _LAYERS:
        out = np.zeros((128, L.C_out), np.float32)
        for m in range(L.n_mms):
            h, cbi, tau, s, b = _mm_geom(L, m)
            lhsT = lhsT_all[h * L.Kmm:h * L.Kmm + L.Kmm,
                            L.w_off:L.w_off + L.Mmm]
            rhs = act[h * L.Kmm:h * L.Kmm + L.Kmm,
                      cbi * FREE:(cbi + 1) * FREE]
            out[s * L.Mmm:(s + 1) * L.Mmm,
                tau * BLK + b * FREE:tau * BLK + (b + 1) * FREE] = lhsT.T @ rhs
        out += biases[:, L.i:L.i + 1]
        act = np.tanh(out) if L.i < N_LAYERS - 1 else out
        act = act.astype(BF16).astype(np.float32)
    return act                                   # [128, 2048]


def unpack_output(per_core_out):
    """[NCORES][NCHUNKS, 128, 2048] device tiles -> [N_POINTS, 3]."""
    ids = replay_ids()
    rows = np.arange(128)
    comp = rows % _LAYERS[-1].out_w_pad
    valid = comp < 3
    n_idx = ids[valid]
    o_idx = np.broadcast_to(comp[valid][:, None], n_idx.shape)
    out = np.empty((N_POINTS, 3), np.float32)
    for core in range(NCORES):
        tiles = per_core_out[core]
        for chunk in range(NCHUNKS):
            base = core * PPC + chunk * CHUNK
            t = np.asarray(tiles[chunk], np.float32)
            out[base + n_idx, o_idx] = t[valid]
    return out


# ---------------------------------------------------------------- bass program

_PROGRAM_CACHE = {}


def _build_program(repeat=1):
    import concourse.bacc as bacc
    import concourse.tile as tile
    from concourse import mybir

    nc = bacc.Bacc("TRN2", target_bir_lowering=False, debug=False,
                   enable_asserts=False, num_devices=NCORES)
    f32 = mybir.dt.float32
    b16 = mybir.dt.bfloat16
    coords_d = nc.dram_tensor("coords", (NCHUNKS, 48, 2048), b16,
                              kind="ExternalInput").ap()
    wh_d = nc.dram_tensor("lhsT_head", (128, W_HEAD), b16,
                          kind="ExternalInput").ap()
    wr_d = nc.dram_tensor("lhsT_rest", (128, W_TOTAL - W_HEAD), b16,
                          kind="ExternalInput").ap()
    b_d = nc.dram_tensor("biases", (128, N_LAYERS), f32,
                         kind="ExternalInput").ap()
    out_d = nc.dram_tensor("out", (NCHUNKS, 128, 2048), b16,
                           kind="ExternalOutput").ap()

    TANH = mybir.ActivationFunctionType.Tanh
    IDENT = mybir.ActivationFunctionType.Identity

    with tile.TileContext(nc) as tc, ExitStack() as ctx:
        wpool = ctx.enter_context(tc.tile_pool(name="weights", bufs=1))
        cpool = ctx.enter_context(tc.tile_pool(name="cin", bufs=6))
        pA = ctx.enter_context(tc.tile_pool(name="a2k", bufs=4))
        pB = ctx.enter_context(tc.tile_pool(name="a4k", bufs=6))
        pC = ctx.enter_context(tc.tile_pool(name="a8k", bufs=4))
        pout = ctx.enter_context(tc.tile_pool(name="aout", bufs=4))
        pspool = ctx.enter_context(
            tc.tile_pool(name="psum", bufs=2, space="PSUM"))

        # PE p-state warmup: the tensor engine needs ~3us of continuous
        # execution to reach full clock. Run throwaway matmuls on a
        # zeroed scratch tile while the input DMAs are in flight so L0
        # hits the PE at full speed. GPSIMD does the memset (every other
        # engine has real work at t0).
        dummy = wpool.tile([128, FREE], b16, tag="warm")
        nc.gpsimd.memset(dummy[:], 0.0)
        wps = pspool.tile([128, FREE], f32, tag="warmps")
        for _ in range(8):
            nc.tensor.matmul(wps[:, 0:FREE], dummy[:, 0:128], dummy[:],
                             start=True, stop=True)

        wt_head = wpool.tile([128, W_HEAD], b16, tag="wth")
        nc.sync.dma_start(out=wt_head[:], in_=wh_d[:])
        bt = wpool.tile([128, N_LAYERS], f32, tag="bt")
        nc.sync.dma_start(out=bt[:], in_=b_d[:])

        ct_of = {}

        def fetch(c):
            t = cpool.tile([48, 2048], b16, tag="cin")
            nc.sync.dma_start(out=t[:], in_=coords_d[c % NCHUNKS])
            ct_of[c] = t

        pool_by_cols = {2048: pA, 4096: pB, 8192: pC}

        def wslice(L, h):
            if L.i < WT_HEAD_LAYERS:
                return wt_head[h * L.Kmm:(h + 1) * L.Kmm,
                               L.w_off:L.w_off + L.Mmm]
            o = L.w_off - W_HEAD
            return wt_rest[h * L.Kmm:(h + 1) * L.Kmm, o:o + L.Mmm]

        def emit_layer(L, act):
            is_last = L.i == N_LAYERS - 1
            pool = pout if is_last else pool_by_cols[L.C_out]
            out_t = pool.tile([128, L.C_out], b16, tag=pool.name)
            for tau in range(L.n_mega):
                ps = pspool.tile([128, BLK], f32, tag="ps")
                for r in range(BANKS * L.stack):
                    m = tau * BANKS * L.stack + r
                    h, cbi, _, s, b = _mm_geom(L, m)
                    nc.tensor.matmul(
                        ps[s * L.Mmm:(s + 1) * L.Mmm,
                           b * FREE:(b + 1) * FREE],
                        wslice(L, h),
                        act[h * L.Kmm:(h + 1) * L.Kmm,
                            cbi * FREE:(cbi + 1) * FREE],
                        start=True, stop=True)
                dst = out_t[:, tau * BLK:(tau + 1) * BLK]
                if is_last:
                    # Final layer bias-add stays on ACT: Identity shares
                    # every tanh table set (no reload), the drain is
                    # faster than DVE's (1.2 vs 0.96 GHz), and keeping
                    # the PSUM WAR chain single-engine avoids exposing a
                    # cross-engine PE->DVE->PE stall on ACT's critical
                    # path.
                    nc.scalar.activation(
                        dst, ps[:], IDENT, bias=bt[:, L.i:L.i + 1])
                else:
                    nc.scalar.activation(
                        dst, ps[:], TANH, bias=bt[:, L.i:L.i + 1])
            return out_t

        def emit_tail(c, acts):
            out_t = emit_layer(_LAYERS[-1], acts[c])
            nc.sync.dma_start(out=out_d[c % NCHUNKS], in_=out_t[:])

        seq = [r * NCHUNKS + c for r in range(repeat) for c in range(NCHUNKS)]
        pairs = list(zip(seq[0::2], seq[1::2]))

        # Coords for the first two pairs land before the weight bulk so
        # L0 can start as early as possible.
        fetch(pairs[0][0])
        fetch(pairs[0][1])
        wt_rest = wpool.tile([128, W_TOTAL - W_HEAD], b16, tag="wtr")
        nc.sync.dma_start(out=wt_rest[:], in_=wr_d[:])
        if len(pairs) > 1:
            fetch(pairs[1][0])
            fetch(pairs[1][1])

        acts = {}
        prev = None
        for pi, (ca, cb) in enumerate(pairs):
            is_last_pair = pi == len(pairs) - 1
            first_li = 0
            if pi == 0:
                acts[ca] = ct_of.pop(ca)
                acts[cb] = ct_of.pop(cb)
            else:
                first_li = 1      # L0 was pre-emitted by the previous pair
            for li in range(first_li, N_LAYERS - 1):
                L = _LAYERS[li]
                for ci, c in enumerate((ca, cb)):
                    acts[c] = emit_layer(L, acts[c])
                    if li == N_LAYERS - 2 and not is_last_pair:
                        # Keep ACT fed across the pair boundary: the
                        # next pair's L0 goes onto the PE queue now.
                        n = pairs[pi + 1][ci]
                        acts[n] = emit_layer(_LAYERS[0], ct_of.pop(n))
                if li == N_LAYERS - 2 and is_last_pair:
                    for c in (ca, cb):
                        emit_tail(c, acts)
                if li == 1:
                    # ACT is busy with L0/L1 tanh here; slot the previous
                    # pair's (ACT-free) final layer into the PE stream now
                    # so pair transitions cost ACT nothing.
                    if prev is not None:
                        for c in prev:
                            emit_tail(c, acts)
                    if pi + 2 < len(pairs):
                        fetch(pairs[pi + 2][0])
                        fetch(pairs[pi + 2][1])
            prev = (ca, cb)

    nc.compile()
    return nc


def get_program(repeat=1):
    key = ("nc", repeat)
    if key not in _PROGRAM_CACHE:
        _PROGRAM_CACHE[key] = _build_program(repeat)
    return _PROGRAM_CACHE[key]


def make_in_maps(coords, Ws, bs):
    cp = pack_coords(coords)
    lhsT_head, lhsT_rest, biases = build_weights(Ws, bs)
    return [{"coords": cp[core], "lhsT_head": lhsT_head,
             "lhsT_rest": lhsT_rest, "biases": biases}
            for core in range(NCORES)]


def kernel(**inputs):
    from concourse.bass_utils import run_bass_kernel_spmd

    coords = np.asarray(inputs["coords"], np.float32)
    Ws = [np.asarray(inputs[f"W{i}"], np.float32) for i in range(N_LAYERS)]
    bs = [np.asarray(inputs[f"b{i}"], np.float32) for i in range(N_LAYERS)]

    nc = get_program()
    in_maps = make_in_maps(coords, Ws, bs)
    res = run_bass_kernel_spmd(nc, in_maps, list(range(NCORES)))
    per_core = [res.results[c]["out"] for c in range(NCORES)]
    full = unpack_output(per_core)
    return (full[:, 0:1], full[:, 1:2], full[:, 2:3])


# revision 12
# speedup vs baseline: 1.2494x; 1.0666x over previous
"""Trainium2 Bass kernel for ClassicPINN forward pass (15-layer tiny MLP, tanh).

Strategy (v2)
-------------
Pure data parallel over 8 NeuronCores (131072 points each); 4 chunks of
32768 points per core. Activations live feature-on-partition in bf16:
the 128 SBUF partitions hold G groups of the layer width, each group a
different 512-point column block. Weights are block-diagonalized on the
host so every matmul is dense [K<=128, M<=128] x [K, 512] -> PSUM fp32.

ACT (ScalarE tanh) is the roofline (~216 tanh elems/point). v2 cuts ACT
waste and idle vs v1:
  * bf16 matmul dataflow (same 1 cyc/row PE speed as fp32r) allows PE
    tile_position, so layers with Mmm=64 (L13, L14) stack 2 matmuls
    partition-wise in one PSUM bank set -> every ACT drains a full
    [128, 2048] tile (v1 ran half-width ACTs for L13).
  * L14 reads the *folded* L13 tile with Gmm=8 (was 4): half the
    matmuls and half the DVE drain of v1.
  * All 15 layers run in interleaved chunk pairs (bf16 halves SBUF so
    the wide layers fit); each pair's L14 (no tanh -> no ACT work) is
    deferred into the next pair's early layers so ACT never waits on a
    pair transition. The final pair interleaves L13/L14 per chunk.
  * Coords prefetched a pair ahead; weight DMA split so L0/L1 weights
    land before the bulk; bf16 output halves the out-DMA.

The same schedule object drives the Bass builder, a numpy simulator
(used by test.py), and an integer "point id" replay that yields the
output unpack permutation.
"""

import numpy as np
from contextlib import ExitStack
import ml_dtypes

BF16 = ml_dtypes.bfloat16

WIDTHS = [3, 8, 8, 8, 8, 8, 8, 8, 16, 16, 16, 32, 32, 32, 16, 3]
N_LAYERS = 15
N_POINTS = 1048576
NCORES = 8
PPC = N_POINTS // NCORES          # 131072 points per core
NCHUNKS = 4
CHUNK = PPC // NCHUNKS            # 32768 points per chunk
FREE = 512                        # matmul moving free dim
BANKS = 4                         # PSUM banks per mega-tile / ACT
BLK = BANKS * FREE                # 2048 cols per mega-tile
WT_HEAD_LAYERS = 2                # layers whose weights ship in the early DMA


class _Layer:
    pass


def _make_schedule():
    """Uniform per-layer schedule. Every layer's input is a physical
    [P_in, C_in] tile holding G_in groups of in_w features per column
    (stacked layers interleave two virtual column blocks row-wise, which
    simply doubles the group count seen by the next layer)."""
    layers = []
    G_in, C_in, P_in = 16, 2048, 48   # coords: 16 groups x 3 feats
    w_off = 0
    for i in range(N_LAYERS):
        in_w, out_w = WIDTHS[i], WIDTHS[i + 1]
        out_w_pad = 8 if out_w == 3 else out_w
        L = _Layer()
        L.i, L.in_w, L.out_w, L.out_w_pad = i, in_w, out_w, out_w_pad
        L.G_in, L.C_in, L.P_in = G_in, C_in, P_in
        L.Gmm = min(G_in, 128 // out_w_pad)
        L.n_half = G_in // L.Gmm
        L.Kmm = L.Gmm * in_w
        L.Mmm = L.Gmm * out_w_pad
        L.stack = 128 // L.Mmm        # matmuls stacked per PSUM bank set
        L.ncb = C_in // FREE
        L.n_mms = L.ncb * L.n_half
        assert L.n_mms % (BANKS * L.stack) == 0, (i, L.n_mms, L.stack)
        L.n_mega = L.n_mms // (BANKS * L.stack)
        L.C_out = L.n_mega * BLK
        L.w_off = w_off
        w_off += L.Mmm
        layers.append(L)
        G_in, C_in, P_in = 128 // out_w_pad, L.C_out, 128
    return layers, w_off


_LAYERS, W_TOTAL = _make_schedule()
W_HEAD = sum(L.Mmm for L in _LAYERS[:WT_HEAD_LAYERS])


def _mm_geom(L, m):
    """Matmul m -> (input half, input col block, mega-tile, stack row, bank)."""
    h, cbi = divmod(m, L.ncb)
    tau, r = divmod(m, BANKS * L.stack)
    s, b = divmod(r, BANKS)
    return h, cbi, tau, s, b


# ---------------------------------------------------------------- host packing

def pack_coords(coords):
    """[N_POINTS, 3] -> bf16 [NCORES, NCHUNKS, 48, 2048] matching L0 layout.

    Per core: point n = chunk*32768 + t*8192 + g*512 + j lives at
    partition g*3+f, column t*512+j of tile [core, chunk].
    """
    c = np.ascontiguousarray(coords, dtype=np.float32)
    c = c.reshape(NCORES, NCHUNKS, 4, 16, FREE, 3)
    c = c.transpose(0, 1, 3, 5, 2, 4)            # core, chunk, g, f, t, j
    return np.ascontiguousarray(c.reshape(NCORES, NCHUNKS, 48, 2048)).astype(BF16)


def build_weights(Ws, bs):
    """Block-diagonal lhsT stacks (bf16 head/rest) and bias matrix [128, 15]."""
    lhsT_all = np.zeros((128, W_TOTAL), np.float32)
    biases = np.zeros((128, N_LAYERS), np.float32)
    for L in _LAYERS:
        W = np.asarray(Ws[L.i], np.float32)      # [out_w, in_w]
        bd = np.zeros((L.Kmm, L.Mmm), np.float32)
        for g in range(L.Gmm):
            bd[g * L.in_w:(g + 1) * L.in_w,
               g * L.out_w_pad:g * L.out_w_pad + L.out_w] = W.T
        for h in range(L.n_half):
            base = h * L.Kmm
            lhsT_all[base:base + L.Kmm, L.w_off:L.w_off + L.Mmm] = bd
        b = np.asarray(bs[L.i], np.float32)
        q = np.arange(128) % L.out_w_pad
        col = np.where(q < L.out_w, b[np.minimum(q, L.out_w - 1)], 0.0)
        biases[:, L.i] = col
    lhsT16 = lhsT_all.astype(BF16)
    return lhsT16[:, :W_HEAD].copy(), lhsT16[:, W_HEAD:].copy(), biases


def replay_ids():
    """Propagate chunk-local point ids through the schedule.

    Returns [128, 2048] int array: element (p, c) of the final output
    tile holds component (p % 8) of chunk-local point ids[p, c].
    """
    ids = np.zeros((48, 2048), np.int64)
    j = np.arange(FREE)
    for g in range(16):
        for t in range(4):
            for f in range(3):
                ids[g * 3 + f, t * FREE:(t + 1) * FREE] = t * 8192 + g * FREE + j
    for L in _LAYERS:
        out = np.zeros((128, L.C_out), np.int64)
        for m in range(L.n_mms):
            h, cbi, tau, s, b = _mm_geom(L, m)
            src = ids[h * L.Kmm:h * L.Kmm + L.Kmm:L.in_w,
                      cbi * FREE:(cbi + 1) * FREE]          # [Gmm, 512]
            out[s * L.Mmm:(s + 1) * L.Mmm,
                tau * BLK + b * FREE:tau * BLK + (b + 1) * FREE] = \
                np.repeat(src, L.out_w_pad, axis=0)
        ids = out
    return ids


def simulate_chunk(coords_tile, lhsT_head, lhsT_rest, biases):
    """Numpy mirror of the device program for one [48, 2048] chunk tile."""
    lhsT_all = np.concatenate([np.asarray(lhsT_head, np.float32),
                               np.asarray(lhsT_rest, np.float32)], axis=1)
    act = np.asarray(coords_tile, np.float32)
    for L in _LAYERS:
        out = np.zeros((128, L.C_out), np.float32)
        for m in range(L.n_mms):
            h, cbi, tau, s, b = _mm_geom(L, m)
            lhsT = lhsT_all[h * L.Kmm:h * L.Kmm + L.Kmm,
                            L.w_off:L.w_off + L.Mmm]
            rhs = act[h * L.Kmm:h * L.Kmm + L.Kmm,
                      cbi * FREE:(cbi + 1) * FREE]
            out[s * L.Mmm:(s + 1) * L.Mmm,
                tau * BLK + b * FREE:tau * BLK + (b + 1) * FREE] = lhsT.T @ rhs
        out += biases[:, L.i:L.i + 1]
        act = np.tanh(out) if L.i < N_LAYERS - 1 else out
        act = act.astype(BF16).astype(np.float32)
    return act                                   # [128, 2048]


def unpack_output(per_core_out):
    """[NCORES][NCHUNKS, 128, 2048] device tiles -> [N_POINTS, 3]."""
    ids = replay_ids()
    rows = np.arange(128)
    comp = rows % _LAYERS[-1].out_w_pad
    valid = comp < 3
    n_idx = ids[valid]
    o_idx = np.broadcast_to(comp[valid][:, None], n_idx.shape)
    out = np.empty((N_POINTS, 3), np.float32)
    for core in range(NCORES):
        tiles = per_core_out[core]
        for chunk in range(NCHUNKS):
            base = core * PPC + chunk * CHUNK
            t = np.asarray(tiles[chunk], np.float32)
            out[base + n_idx, o_idx] = t[valid]
    return out


# ---------------------------------------------------------------- bass program

_PROGRAM_CACHE = {}


def _build_program(repeat=1):
    import concourse.bacc as bacc
    import concourse.tile as tile
    from concourse import mybir

    nc = bacc.Bacc("TRN2", target_bir_lowering=False, debug=False,
                   enable_asserts=False, num_devices=NCORES)
    f32 = mybir.dt.float32
    b16 = mybir.dt.bfloat16
    coords_d = nc.dram_tensor("coords", (NCHUNKS, 48, 2048), b16,
                              kind="ExternalInput").ap()
    wh_d = nc.dram_tensor("lhsT_head", (128, W_HEAD), b16,
                          kind="ExternalInput").ap()
    wr_d = nc.dram_tensor("lhsT_rest", (128, W_TOTAL - W_HEAD), b16,
                          kind="ExternalInput").ap()
    b_d = nc.dram_tensor("biases", (128, N_LAYERS), f32,
                         kind="ExternalInput").ap()
    out_d = nc.dram_tensor("out", (NCHUNKS, 128, 2048), b16,
                           kind="ExternalOutput").ap()

    TANH = mybir.ActivationFunctionType.Tanh
    IDENT = mybir.ActivationFunctionType.Identity

    with tile.TileContext(nc) as tc, ExitStack() as ctx:
        wpool = ctx.enter_context(tc.tile_pool(name="weights", bufs=1))
        cpool = ctx.enter_context(tc.tile_pool(name="cin", bufs=6))
        pA = ctx.enter_context(tc.tile_pool(name="a2k", bufs=4))
        pB = ctx.enter_context(tc.tile_pool(name="a4k", bufs=6))
        pC = ctx.enter_context(tc.tile_pool(name="a8k", bufs=4))
        pout = ctx.enter_context(tc.tile_pool(name="aout", bufs=4))
        pspool = ctx.enter_context(
            tc.tile_pool(name="psum", bufs=2, space="PSUM"))

        # PE p-state warmup: the tensor engine needs ~3us of continuous
        # execution to reach full clock. Run throwaway matmuls on a
        # zeroed scratch tile while the input DMAs are in flight so L0
        # hits the PE at full speed. GPSIMD does the memset (every other
        # engine has real work at t0).
        dummy = wpool.tile([128, FREE], b16, tag="warm")
        nc.gpsimd.memset(dummy[:], 0.0)
        wps = pspool.tile([128, BLK], f32, tag="ps")
        for _ in range(8):
            nc.tensor.matmul(wps[:, 0:FREE], dummy[:, 0:128], dummy[:],
                             start=True, stop=True)

        wt_head = wpool.tile([128, W_HEAD], b16, tag="wth")
        nc.sync.dma_start(out=wt_head[:], in_=wh_d[:])
        bt = wpool.tile([128, N_LAYERS], f32, tag="bt")
        nc.sync.dma_start(out=bt[:], in_=b_d[:])

        ct_of = {}

        def fetch(c):
            t = cpool.tile([48, 2048], b16, tag="cin")
            nc.sync.dma_start(out=t[:], in_=coords_d[c % NCHUNKS])
            ct_of[c] = t

        pool_by_cols = {2048: pA, 4096: pB, 8192: pC}

        def wslice(L, h):
            if L.i < WT_HEAD_LAYERS:
                return wt_head[h * L.Kmm:(h + 1) * L.Kmm,
                               L.w_off:L.w_off + L.Mmm]
            o = L.w_off - W_HEAD
            return wt_rest[h * L.Kmm:(h + 1) * L.Kmm, o:o + L.Mmm]

        def emit_layer(L, act):
            is_last = L.i == N_LAYERS - 1
            pool = pout if is_last else pool_by_cols[L.C_out]
            out_t = pool.tile([128, L.C_out], b16, tag=pool.name)
            for tau in range(L.n_mega):
                ps = pspool.tile([128, BLK], f32, tag="ps")
                for r in range(BANKS * L.stack):
                    m = tau * BANKS * L.stack + r
                    h, cbi, _, s, b = _mm_geom(L, m)
                    nc.tensor.matmul(
                        ps[s * L.Mmm:(s + 1) * L.Mmm,
                           b * FREE:(b + 1) * FREE],
                        wslice(L, h),
                        act[h * L.Kmm:(h + 1) * L.Kmm,
                            cbi * FREE:(cbi + 1) * FREE],
                        start=True, stop=True)
                dst = out_t[:, tau * BLK:(tau + 1) * BLK]
                if is_last:
                    # Final layer bias-add stays on ACT: Identity shares
                    # every tanh table set (no reload), the drain is
                    # faster than DVE's (1.2 vs 0.96 GHz), and keeping
                    # the PSUM WAR chain single-engine avoids exposing a
                    # cross-engine PE->DVE->PE stall on ACT's critical
                    # path.
                    nc.scalar.activation(
                        dst, ps[:], IDENT, bias=bt[:, L.i:L.i + 1])
                else:
                    nc.scalar.activation(
                        dst, ps[:], TANH, bias=bt[:, L.i:L.i + 1])
            return out_t

        def emit_tail(c, acts, split=False):
            if not split:
                out_t = emit_layer(_LAYERS[-1], acts[c])
                nc.sync.dma_start(out=out_d[c % NCHUNKS], in_=out_t[:])
                return
            # Program-final tails: drain the single L14 mega in halves
            # into separate tiles so the first half's store DMA overlaps
            # the second half's drain.
            L = _LAYERS[-1]
            ps = pspool.tile([128, BLK], f32, tag="ps")
            for m in range(BANKS * L.stack):
                h, cbi, _, s, b = _mm_geom(L, m)
                nc.tensor.matmul(
                    ps[s * L.Mmm:(s + 1) * L.Mmm, b * FREE:(b + 1) * FREE],
                    wslice(L, h),
                    acts[c][h * L.Kmm:(h + 1) * L.Kmm,
                            cbi * FREE:(cbi + 1) * FREE],
                    start=True, stop=True)
            half = BLK // 2
            for j in range(2):
                ot = pout.tile([128, half], b16, tag="aout")
                nc.scalar.activation(ot[:], ps[:, j * half:(j + 1) * half],
                                     IDENT, bias=bt[:, L.i:L.i + 1])
                nc.sync.dma_start(
                    out=out_d[c % NCHUNKS][:, j * half:(j + 1) * half],
                    in_=ot[:])

        seq = [r * NCHUNKS + c for r in range(repeat) for c in range(NCHUNKS)]
        pairs = list(zip(seq[0::2], seq[1::2]))

        # Coords for the first two pairs land before the weight bulk so
        # L0 can start as early as possible.
        fetch(pairs[0][0])
        fetch(pairs[0][1])
        wt_rest = wpool.tile([128, W_TOTAL - W_HEAD], b16, tag="wtr")
        nc.sync.dma_start(out=wt_rest[:], in_=wr_d[:])
        if len(pairs) > 1:
            fetch(pairs[1][0])
            fetch(pairs[1][1])

        acts = {}
        prev = None
        for pi, (ca, cb) in enumerate(pairs):
            is_last_pair = pi == len(pairs) - 1
            first_li = 0
            if pi == 0:
                acts[ca] = ct_of.pop(ca)
                acts[cb] = ct_of.pop(cb)
            else:
                first_li = 1      # L0 was pre-emitted by the previous pair
            for li in range(first_li, N_LAYERS - 1):
                L = _LAYERS[li]
                for ci, c in enumerate((ca, cb)):
                    acts[c] = emit_layer(L, acts[c])
                    if li == N_LAYERS - 2 and not is_last_pair:
                        # Keep ACT fed across the pair boundary: the
                        # next pair's L0 goes onto the PE queue now.
                        n = pairs[pi + 1][ci]
                        acts[n] = emit_layer(_LAYERS[0], ct_of.pop(n))
                if li == N_LAYERS - 2 and is_last_pair:
                    for c in (ca, cb):
                        emit_tail(c, acts, split=True)
                if li == 1:
                    # ACT is busy with L0/L1 tanh here; slot the previous
                    # pair's (ACT-free) final layer into the PE stream now
                    # so pair transitions cost ACT nothing.
                    if prev is not None:
                        for c in prev:
                            emit_tail(c, acts)
                    if pi + 2 < len(pairs):
                        fetch(pairs[pi + 2][0])
                        fetch(pairs[pi + 2][1])
            prev = (ca, cb)

    nc.compile()
    return nc


def get_program(repeat=1):
    key = ("nc", repeat)
    if key not in _PROGRAM_CACHE:
        _PROGRAM_CACHE[key] = _build_program(repeat)
    return _PROGRAM_CACHE[key]


def make_in_maps(coords, Ws, bs):
    cp = pack_coords(coords)
    lhsT_head, lhsT_rest, biases = build_weights(Ws, bs)
    return [{"coords": cp[core], "lhsT_head": lhsT_head,
             "lhsT_rest": lhsT_rest, "biases": biases}
            for core in range(NCORES)]


def kernel(**inputs):
    from concourse.bass_utils import run_bass_kernel_spmd

    coords = np.asarray(inputs["coords"], np.float32)
    Ws = [np.asarray(inputs[f"W{i}"], np.float32) for i in range(N_LAYERS)]
    bs = [np.asarray(inputs[f"b{i}"], np.float32) for i in range(N_LAYERS)]

    nc = get_program()
    in_maps = make_in_maps(coords, Ws, bs)
    res = run_bass_kernel_spmd(nc, in_maps, list(range(NCORES)))
    per_core = [res.results[c]["out"] for c in range(NCORES)]
    full = unpack_output(per_core)
    return (full[:, 0:1], full[:, 1:2], full[:, 2:3])
